# revision 5
# baseline (speedup 1.0000x reference)
"""Trainium2 Bass kernel for CompoundWordAutoregressiveWrapper loss_fn.

PROBE build: SWDGE cast-DMA loads (fp32 DRAM -> bf16 SBUF). If the
~427 GB/s streaming cap is on the SBUF-write side, halving SBUF bytes
lets the HBM read run faster; if it is read-side, this matches v2.

Computes 8 scalar losses (7 masked-mean CE + 1 input-only MSE); see
_combine for the host epilogue.
"""

import sys

if "/opt/trn_rl_repo" not in sys.path:
    sys.path.insert(0, "/opt/trn_rl_repo")

import numpy as np

_B, _S = 2, 1024
_P = _B * _S  # 2048 flattened rows
_V = 6913
_NCORES = 8
_ROWS = _P // _NCORES  # 256 rows per core
_HEADS = (
    "proj_type",
    "proj_barbeat",
    "proj_tempo",
    "proj_instrument",
    "proj_note_name",
    "proj_octave",
    "proj_duration",
)
_NHEADS = len(_HEADS)
_NTILES = _ROWS // 128  # 2
_NITER = _NTILES * _NHEADS  # 14

# Decreasing chunk sizes for the last tile.
_TAIL = (2465, 1745, 1235, 874, 594)
assert sum(_TAIL) == _V

# outb columns: [0:14] per-iteration sumexp (col = t*7+h; last iteration's
# chunk 0 in col 13), [14:18] last-iteration chunks 1..4, [18:32] gathers.
_GCOL = _NITER + len(_TAIL) - 1  # 18
_NOUT = _GCOL + _NITER  # 32

_F0 = 1.6023243915697094e-05

_PROGRAM_CACHE = {}


def _build(rows=_ROWS, v=_V):
    """Build the SPMD Bass program for one core: rows x v per head."""
    import concourse.bass as bass
    import concourse.mybir as mybir
    from concourse import bacc, tile

    f32 = mybir.dt.float32
    i32 = mybir.dt.int32
    bf16 = mybir.dt.bfloat16
    AF = mybir.ActivationFunctionType

    assert rows % 128 == 0
    ntiles = rows // 128

    nc = bacc.Bacc(trn_type="TRN2")
    lg_dram = [
        nc.dram_tensor(f"lg{h}", [rows * v], f32, kind="ExternalInput")
        for h in range(_NHEADS)
    ]
    goff_dram = nc.dram_tensor("goff", [rows, 8], i32, kind="ExternalInput")
    out_dram = nc.dram_tensor("out", [128, _NOUT], f32, kind="ExternalOutput")

    lg2d = [d.rearrange("(r c) -> r c", c=v) for d in lg_dram]
    lgflat = [d.rearrange("(n o) -> n o", o=1) for d in lg_dram]

    def ranges(sizes):
        out, a = [], 0
        for s in sizes:
            out.append((a, a + s))
            a += s
        return out

    tail_r = ranges(_TAIL)

    with tile.TileContext(nc) as tc:
        with (
            tc.tile_pool(name="lg", bufs=10) as lgp,
            tc.tile_pool(name="es", bufs=1) as esp,
            tc.tile_pool(name="sm", bufs=1) as smp,
        ):
            goff = []
            for t in range(ntiles):
                g = smp.tile([128, 8], i32, tag=f"goff{t}")
                nc.gpsimd.dma_start(g[:], goff_dram[t * 128 : (t + 1) * 128, :])
                goff.append(g)
            outb = smp.tile([128, _NOUT], f32, tag="outb")

            for h in range(_NHEADS):
                for t in range(ntiles):
                    col = t * _NHEADS + h
                    last = h == _NHEADS - 1 and t == ntiles - 1
                    # bf16 tile: the SWDGE load casts fp32 DRAM -> bf16 SBUF
                    lg = lgp.tile([128, v], bf16, tag="lg")
                    src = lg2d[h][t * 128 : (t + 1) * 128, :]
                    es = esp.tile([128, v], bf16, tag="es")
                    if not last:
                        nc.gpsimd.dma_start(lg[:], src)
                        nc.scalar.activation(
                            es[:],
                            lg[:],
                            AF.Exp,
                            accum_out=outb[:, col : col + 1],
                        )
                    else:
                        for a, b in tail_r:
                            nc.gpsimd.dma_start(lg[:, a:b], src[:, a:b])
                        for ci, (a, b) in enumerate(tail_r):
                            cc = col if ci == 0 else _NITER + ci - 1
                            nc.scalar.activation(
                                es[:, a:b],
                                lg[:, a:b],
                                AF.Exp,
                                accum_out=outb[:, cc : cc + 1],
                            )

            for h in range(_NHEADS):
                for t in range(ntiles):
                    gc = _GCOL + t * _NHEADS + h
                    nc.gpsimd.indirect_dma_start(
                        out=outb[:, gc : gc + 1],
                        out_offset=None,
                        in_=lgflat[h][:],
                        in_offset=bass.IndirectOffsetOnAxis(
                            ap=goff[t][:, h : h + 1], axis=0
                        ),
                    )

            nc.sync.dma_start(out_dram[:], outb[:])

    return nc


def _get_program():
    if "nc" not in _PROGRAM_CACHE:
        nc = _build()
        nc.finalize()
        _PROGRAM_CACHE["nc"] = nc
    return _PROGRAM_CACHE["nc"]


def _make_in_maps(inputs):
    heads = [
        np.ascontiguousarray(np.asarray(inputs[n], dtype=np.float32)).reshape(_P * _V)
        for n in _HEADS
    ]
    x = np.asarray(inputs["x"])
    tgt = x[:, 1:, :].reshape(_P, 12)
    goff = np.zeros((_P, 8), np.int32)
    rloc = (np.arange(_P, dtype=np.int64) % _ROWS) * _V
    for h in range(_NHEADS):
        goff[:, h] = (rloc + tgt[:, h].astype(np.int64)).astype(np.int32)
    in_maps = []
    for c in range(_NCORES):
        sl = slice(c * _ROWS, (c + 1) * _ROWS)
        fl = slice(c * _ROWS * _V, (c + 1) * _ROWS * _V)
        m = {f"lg{h}": heads[h][fl] for h in range(_NHEADS)}
        m["goff"] = goff[sl]
        in_maps.append(m)
    return in_maps


def _combine(core_outs, x):
    """core_outs: [ncores, 128, _NOUT] -> [8] float32 losses."""
    o = np.asarray(core_outs, dtype=np.float64)  # [C, 128, _NOUT]
    sumexp = o[:, :, 0:_NITER].copy()
    sumexp[:, :, _NITER - 1] += o[:, :, _NITER : _NITER + len(_TAIL) - 1].sum(axis=2)
    picked = o[:, :, _GCOL : _GCOL + _NITER]
    lse = np.log(sumexp).reshape(_NCORES, 128, _NTILES, _NHEADS)
    pick = picked.reshape(_NCORES, 128, _NTILES, _NHEADS)
    nll = (lse - pick).transpose(0, 2, 1, 3).reshape(_P, _NHEADS)

    tgt = np.asarray(x)[:, 1:, :].reshape(_P, 12)
    mask = (tgt[:, 0] != 0).astype(np.float64)
    tot = mask.sum()
    if tot == 0.0:
        return np.zeros(8, np.float32)
    ce = (nll * mask[:, None]).sum(axis=0) / tot
    t11 = tgt[:, 11].astype(np.float64)
    mse = (mask * (t11 - _F0) ** 2).sum() / tot
    return np.concatenate([ce, [mse]]).astype(np.float32)


def _execute(inputs, trace=False, **kwargs):
    from concourse import bass_utils

    nc = _get_program()
    in_maps = _make_in_maps(inputs)
    res = bass_utils.run_bass_kernel_spmd(
        nc, in_maps, core_ids=list(range(_NCORES)), trace=trace, **kwargs
    )
    core_outs = np.stack([np.asarray(r["out"]) for r in res.results])
    return _combine(core_outs, inputs["x"]), res


def kernel(**inputs) -> np.ndarray:
    out, _ = _execute(inputs)
    return out


# revision 6
# speedup vs baseline: 1.3659x; 1.3659x over previous
"""Trainium2 Bass kernel for CompoundWordAutoregressiveWrapper loss_fn.

Computes 8 scalar losses:
  - 7 masked-mean cross-entropy losses, one per projection head
    ([2,1024,6913] logits each), target channels 0..6 of x[:,1:,:],
    mask = (x[:,1:,0] != 0).
  - 1 masked-mean MSE between a constant f0 (the "temps" branch of the
    reference constant-folds: softmax over an axis of size 1 is
    identically 1.0, so f is input-independent) and x[:,1:,11].

Strategy (data-parallel, per sharding hint): flatten p = B*S = 2048 rows,
shard 256 rows to each of 8 NeuronCores. Each core:
  - streams its 7x[256,6913] logit slices from HBM once, ALL on the
    sync-engine HWDGE ring (a single ring drives all 16 SDMA engines at
    ~420 GB/s fabric rate; the sync queue does nothing else, so pool
    recycling waits never block compute);
  - each [128,6913] tile arrives as two chunks [4608,2305]; ScalarE runs
    a plain Exp per chunk (no accumulator read on the critical path) and
    the idle VectorE does the row-sum of the bf16 exp tile. The scalar
    engine's clock varies run-to-run (~1.0-1.2 GHz), so per-tile ACT work
    (~7.7us worst case) is budgeted under the ~8.3us DMA cadence;
  - the final tile is split into decreasing chunks [2993,2240,1680] (the
    last one using the ScalarE accumulator directly) so the exposed
    compute after the last byte lands is small;
  - logits[row, target[row]] is fetched by indirect (gather) DMA from
    DRAM via SWDGE using host-precomputed flat element offsets;
  - one [128, 30] tile (16 sumexp cols + 14 gathered-logit cols) is
    DMA'd out; the O(rows) epilogue (log, masked sums, the input-only
    MSE term, and the cross-core scalar all-reduce) runs on the host
    during unsharding.
"""

import sys

if "/opt/trn_rl_repo" not in sys.path:
    sys.path.insert(0, "/opt/trn_rl_repo")

import numpy as np

_B, _S = 2, 1024
_P = _B * _S  # 2048 flattened rows
_V = 6913
_NCORES = 8
_ROWS = _P // _NCORES  # 256 rows per core
_HEADS = (
    "proj_type",
    "proj_barbeat",
    "proj_tempo",
    "proj_instrument",
    "proj_note_name",
    "proj_octave",
    "proj_duration",
)
_NHEADS = len(_HEADS)
_NTILES = _ROWS // 128  # 2
_NITER = _NTILES * _NHEADS  # 14

# Body tiles stream as two chunks so the scalar engine can start each
# tile's exp before the whole tile lands; the last tile uses decreasing
# chunks so the final exposed exp after the last byte is small.
_BODY = (4608, 2305)
_TAIL = (2993, 2240, 1680)
assert sum(_BODY) == _V and sum(_TAIL) == _V

# outb columns: [0:14] per-iteration sumexp (col = t*7+h; the last
# iteration's chunk 0 lands in col 13), [14:16] last-iteration chunks
# 1..2, [16:30] gathered logits (col 16 + t*7+h).
_GCOL = _NITER + len(_TAIL) - 1  # 16
_NOUT = _GCOL + _NITER  # 30

# f = (s @ d)/6 with s identically 6.0 -> f[...,0] = column sum of
# sin(1*ang) over the 6912-entry trig table; mathematically ~0, fp
# residual ~1.6e-5 (impact on the MSE is ~4e-8 relative).
_F0 = 1.6023243915697094e-05

_PROGRAM_CACHE = {}


def _build(rows=_ROWS, v=_V):
    """Build the SPMD Bass program for one core: rows x v per head."""
    import concourse.bass as bass
    import concourse.mybir as mybir
    from concourse import bacc, tile

    f32 = mybir.dt.float32
    i32 = mybir.dt.int32
    bf16 = mybir.dt.bfloat16
    AF = mybir.ActivationFunctionType

    assert rows % 128 == 0
    ntiles = rows // 128

    # Bacc (not plain Bass): its compile() legalizes multi-wait sync via
    # InstEventSemaphore -- TRN2 compute instructions encode at most 1 wait.
    nc = bacc.Bacc(trn_type="TRN2")
    # 1-D logits tensors: the flat view is what the gather DMA indexes into;
    # the streaming loads re-view them as [rows, v].
    lg_dram = [
        nc.dram_tensor(f"lg{h}", [rows * v], f32, kind="ExternalInput")
        for h in range(_NHEADS)
    ]
    # goff[r, h] = r*v + target[r, h]: flat element offsets for the gather
    goff_dram = nc.dram_tensor("goff", [rows, 8], i32, kind="ExternalInput")
    out_dram = nc.dram_tensor("out", [128, _NOUT], f32, kind="ExternalOutput")

    lg2d = [d.rearrange("(r c) -> r c", c=v) for d in lg_dram]
    # [N, 1] view for the gather: offsets index axis 0, one element each
    lgflat = [d.rearrange("(n o) -> n o", o=1) for d in lg_dram]

    def ranges(sizes):
        out, a = [], 0
        for s in sizes:
            out.append((a, a + s))
            a += s
        return out

    body_r = ranges(_BODY)
    tail_r = ranges(_TAIL)

    with tile.TileContext(nc) as tc:
        with (
            tc.tile_pool(name="lg", bufs=6) as lgp,
            tc.tile_pool(name="es", bufs=2) as esp,
            tc.tile_pool(name="sm", bufs=1) as smp,
        ):
            # small loads on SWDGE so the sync HWDGE ring starts with the
            # big streaming loads
            goff = []
            for t in range(ntiles):
                g = smp.tile([128, 8], i32, tag=f"goff{t}")
                nc.gpsimd.dma_start(g[:], goff_dram[t * 128 : (t + 1) * 128, :])
                goff.append(g)
            outb = smp.tile([128, _NOUT], f32, tag="outb")

            for h in range(_NHEADS):
                for t in range(ntiles):
                    col = t * _NHEADS + h
                    last = h == _NHEADS - 1 and t == ntiles - 1
                    lg = lgp.tile([128, v], f32, tag="lg")
                    src = lg2d[h][t * 128 : (t + 1) * 128, :]
                    # exp output is consumed by VectorE's row-sum; bf16
                    # halves both the write and the reduce-read traffic
                    es = esp.tile([128, v], bf16, tag="es")
                    if not last:
                        for a, b in body_r:
                            nc.sync.dma_start(lg[:, a:b], src[:, a:b])
                            nc.scalar.activation(es[:, a:b], lg[:, a:b], AF.Exp)
                        nc.vector.reduce_sum(
                            outb[:, col : col + 1],
                            es[:],
                            axis=mybir.AxisListType.X,
                        )
                    else:
                        for a, b in tail_r:
                            nc.sync.dma_start(lg[:, a:b], src[:, a:b])
                        for ci, (a, b) in enumerate(tail_r):
                            cc = col if ci == 0 else _NITER + ci - 1
                            if ci < len(tail_r) - 1:
                                nc.scalar.activation(es[:, a:b], lg[:, a:b], AF.Exp)
                                nc.vector.reduce_sum(
                                    outb[:, cc : cc + 1],
                                    es[:, a:b],
                                    axis=mybir.AxisListType.X,
                                )
                            else:
                                # final chunk: ScalarE accumulator avoids a
                                # trailing VectorE pass on the critical tail
                                nc.scalar.activation(
                                    es[:, a:b],
                                    lg[:, a:b],
                                    AF.Exp,
                                    accum_out=outb[:, cc : cc + 1],
                                )

            # gather DMAs: one per (head, row-tile), indexing DRAM directly;
            # tiny SWDGE traffic fully overlapped with the streaming loads
            for h in range(_NHEADS):
                for t in range(ntiles):
                    gc = _GCOL + t * _NHEADS + h
                    nc.gpsimd.indirect_dma_start(
                        out=outb[:, gc : gc + 1],
                        out_offset=None,
                        in_=lgflat[h][:],
                        in_offset=bass.IndirectOffsetOnAxis(
                            ap=goff[t][:, h : h + 1], axis=0
                        ),
                    )

            nc.sync.dma_start(out_dram[:], outb[:])

    return nc


def _get_program():
    if "nc" not in _PROGRAM_CACHE:
        nc = _build()
        nc.finalize()
        _PROGRAM_CACHE["nc"] = nc
    return _PROGRAM_CACHE["nc"]


def _make_in_maps(inputs):
    heads = [
        np.ascontiguousarray(np.asarray(inputs[n], dtype=np.float32)).reshape(_P * _V)
        for n in _HEADS
    ]
    x = np.asarray(inputs["x"])
    tgt = x[:, 1:, :].reshape(_P, 12)
    goff = np.zeros((_P, 8), np.int32)
    rloc = (np.arange(_P, dtype=np.int64) % _ROWS) * _V
    for h in range(_NHEADS):
        goff[:, h] = (rloc + tgt[:, h].astype(np.int64)).astype(np.int32)
    in_maps = []
    for c in range(_NCORES):
        sl = slice(c * _ROWS, (c + 1) * _ROWS)
        fl = slice(c * _ROWS * _V, (c + 1) * _ROWS * _V)
        m = {f"lg{h}": heads[h][fl] for h in range(_NHEADS)}
        m["goff"] = goff[sl]
        in_maps.append(m)
    return in_maps


def _combine(core_outs, x):
    """core_outs: [ncores, 128, _NOUT] -> [8] float32 losses.

    Host epilogue: masked sums across rows, the input-only MSE term, and
    the cross-core scalar reduction.
    """
    o = np.asarray(core_outs, dtype=np.float64)  # [C, 128, _NOUT]
    sumexp = o[:, :, 0:_NITER].copy()
    # fold the last iteration's extra chunk columns into its primary col
    sumexp[:, :, _NITER - 1] += o[:, :, _NITER : _NITER + len(_TAIL) - 1].sum(axis=2)
    picked = o[:, :, _GCOL : _GCOL + _NITER]
    # [C, 128, t, h] -> flat row r = c*ROWS + t*128 + p
    lse = np.log(sumexp).reshape(_NCORES, 128, _NTILES, _NHEADS)
    pick = picked.reshape(_NCORES, 128, _NTILES, _NHEADS)
    nll = (lse - pick).transpose(0, 2, 1, 3).reshape(_P, _NHEADS)

    tgt = np.asarray(x)[:, 1:, :].reshape(_P, 12)
    mask = (tgt[:, 0] != 0).astype(np.float64)
    tot = mask.sum()
    if tot == 0.0:
        return np.zeros(8, np.float32)
    ce = (nll * mask[:, None]).sum(axis=0) / tot
    t11 = tgt[:, 11].astype(np.float64)
    mse = (mask * (t11 - _F0) ** 2).sum() / tot
    return np.concatenate([ce, [mse]]).astype(np.float32)


def _execute(inputs, trace=False, **kwargs):
    from concourse import bass_utils

    nc = _get_program()
    in_maps = _make_in_maps(inputs)
    res = bass_utils.run_bass_kernel_spmd(
        nc, in_maps, core_ids=list(range(_NCORES)), trace=trace, **kwargs
    )
    core_outs = np.stack([np.asarray(r["out"]) for r in res.results])
    return _combine(core_outs, inputs["x"]), res


def kernel(**inputs) -> np.ndarray:
    out, _ = _execute(inputs)
    return out
